# revision 10
# baseline (speedup 1.0000x reference)
"""GPT2Attention Trainium2 Bass kernel.

Problem: B=2, S=2048, E=1024, H=16 heads, D=64.
  qkv = x @ c_attn_w + c_attn_b; causal softmax attention; out @ c_proj_w + c_proj_b.

Sharding: 8 cores = 2 (batch) x 4 (head-groups of 4 heads).  Each core computes
its batch's attention for its 4 heads plus the partial c_proj contribution
(rows of c_proj_w belonging to its heads).  Host sums the 4 partials per batch
and adds the bias terms (v-bias folds through attention: attn rows sum to 1).

Device pipeline (chunked over 512-query blocks so the tile scheduler can
overlap the PE-heavy projections with the ACT-heavy softmax):
  A(c): QKV projection for query chunk c.  qT/kT per head-pair [128, S]
        (partitions = 2 heads x 64 dims); v natural [tokens, 4 heads, 65]
        with a ones column for the softmax denominator.
  B(c): per head: scoresT tiles [128 keys, 512 q] on PE (trimmed to the
        causal boundary), exp on ACT, diagonal boundary zeroed via
        copy_predicated on DVE.  attn@v runs transposed: po[128 q, 65]
        accumulates over key tiles (cost = 65 rows/matmul instead of 512).
        Normalize by the ones-column denominator -> c2 [128 tok, 128 dims].
  T(c): DMA-transpose c2 -> cT [dims, tokens] (idle DMA hardware).
  C(c): partial c_proj from cT; evict PSUM->SBUF split ACT/DVE; DMA y out.
"""

from contextlib import ExitStack

import numpy as np
import ml_dtypes

import bass_rust
import concourse.bass as bass
import concourse.tile as tile
from concourse import mybir
from concourse import bass_utils


def _patched_drain_and_barrier(self, tick_clock, wait_clock):
    # The stock walrus in this container rejects instructions carrying more
    # than one sync wait ("Too many sync wait commands" on the kernel-tail
    # Drain).  Spread the final waits across single-wait NOPs instead.
    nc = self.nc
    probe = nc.sync.nop()
    wait_clock.add_sem_waits(
        probe.ins, bass_rust.ScopedClock({None: tick_clock.global_clock}))
    si = probe.ins.sync_info
    waits = list(si.on_wait) if si is not None else []
    if len(waits) > 1:
        probe.ins.sync_info = mybir.SyncInfo(
            on_wait=waits[:1], on_update=list(si.on_update))
        for w in waits[1:]:
            n = nc.sync.nop()
            n.ins.sync_info = mybir.SyncInfo(on_wait=[w], on_update=[])
    nc.sync.drain()
    nc.all_engine_barrier()
    assert self.sems is not None
    popped = nc._tile_sem_poison_stack.pop()
    assert popped is self._sem_poison
    nc.clear_and_free_semaphores(list(self.sems.allocated().values()))
    nc.all_engine_barrier()


tile.TileContext._drain_and_barrier = _patched_drain_and_barrier

_split_ctr = [0]


def _split_sync_waits(nc):
    """Stock walrus allows one sync wait per instruction; hoist extras onto
    single-wait NOPs inserted just before, on the same (in-order) engine."""
    for fn in nc.m.functions:
        for bb in fn.blocks:
            insts = bb.instructions
            out = []
            changed = False
            for inst in insts:
                si = getattr(inst, "sync_info", None)
                waits = list(si.on_wait) if si is not None else []
                if len(waits) > 1:
                    changed = True
                    for w in waits[:-1]:
                        _split_ctr[0] += 1
                        nop = bass_rust.InstNoOp(
                            name=f"I-syncsplit-{_split_ctr[0]}",
                            engine=inst.engine)
                        nop.sync_info = mybir.SyncInfo(on_wait=[w], on_update=[])
                        out.append(nop)
                    inst.sync_info = mybir.SyncInfo(
                        on_wait=[waits[-1]], on_update=list(si.on_update))
                out.append(inst)
            if changed:
                bb.instructions = out

B, S, E, H, D = 2, 2048, 1024, 16, 64
NCORES = 8
HG = 4                # head-group cores per batch
LH = H // HG          # 4 local heads per core
LC = LH * D           # 256 local c_proj rows
NPAIR = LH // 2       # 2 head-pairs per core
P = 128
KT = E // P           # 8 contraction tiles for the projections
QCHUNK = 512
NQC = S // QCHUNK     # 4 query chunks
NKT = S // P          # 16 key tiles
NTT = S // P          # 16 token tiles

FP = mybir.dt.float32
BF = mybir.dt.bfloat16
EXP = mybir.ActivationFunctionType.Exp
COPY = mybir.ActivationFunctionType.Copy


def _build_module():
    nc = bass.Bass("TRN2", target_bir_lowering=False, debug=False,
                   num_devices=NCORES)
    xT = nc.dram_tensor("xT", [E, S], BF, kind="ExternalInput").ap()
    wq = nc.dram_tensor("wq", [E, LC], BF, kind="ExternalInput").ap()
    wk = nc.dram_tensor("wk", [E, LC], BF, kind="ExternalInput").ap()
    wv = nc.dram_tensor("wv", [E, LC], BF, kind="ExternalInput").ap()
    w2 = nc.dram_tensor("w2", [LC, E], BF, kind="ExternalInput").ap()
    tri = nc.dram_tensor("tri", [P, 3 * P], mybir.dt.uint8,
                         kind="ExternalInput").ap()
    bqk = nc.dram_tensor("bqk", [P, 2 * NPAIR], FP, kind="ExternalInput").ap()
    y = nc.dram_tensor("y", [S, E], FP, kind="ExternalOutput").ap()

    with tile.TileContext(nc) as tc:
        _body(tc, xT, wq, wk, wv, w2, tri, bqk, y)
    _split_sync_waits(nc)
    return nc


def _body(tc, xT, wq, wk, wv, w2, tri, bqk, y):
    nc = tc.nc
    ex = ExitStack()
    with ex:
        persist = ex.enter_context(tc.tile_pool(name="persist", bufs=1))

        # ---- persistent tiles ----
        qT2 = [persist.tile([P, S], BF, name=f"qT2_{p}") for p in range(NPAIR)]
        kT2 = [persist.tile([P, S], BF, name=f"kT2_{p}") for p in range(NPAIR)]
        # v natural layout: [token-partitions, ttile, head, dim(+denom col)]
        vall = persist.tile([P, NTT, LH, D + 1], BF, name="vall")
        cT = [persist.tile([P, S], BF, name=f"cT_{p}") for p in range(NPAIR)]
        xt_sb = persist.tile([P, KT, S], BF, name="xt_sb")
        wq_sb = persist.tile([P, KT, LC], BF, name="wq_sb")
        wk_sb = persist.tile([P, KT, LC], BF, name="wk_sb")
        wv_sb = persist.tile([P, KT, LC], BF, name="wv_sb")
        w2_sb = persist.tile([P, 2, E], BF, name="w2_sb")
        # masking predicates for the causal boundary: cols 0:128 = (j < k),
        # 128:384 = (j < 128+k)
        tri_sb = persist.tile([P, 3 * P], mybir.dt.uint8, name="tri_sb")
        zero_sb = persist.tile([P, 2 * P], BF, name="zero_sb")
        bqk_sb = persist.tile([P, 2 * NPAIR], FP, name="bqk_sb")

        nc.vector.memset(zero_sb[:], 0.0)
        nc.vector.memset(vall[:, :, :, D:D + 1], 1.0)

        # ---- input DMAs (ordered so chunk-0 work can start early) ----
        xTr = xT.rearrange("(k p) s -> k p s", p=P)
        nc.sync.dma_start(out=xt_sb[:, :, 0:QCHUNK], in_=xTr[:, :, 0:QCHUNK].rearrange("k p s -> p k s"))
        nc.scalar.dma_start(out=wq_sb[:], in_=wq.rearrange("(k p) c -> p k c", p=P))
        nc.scalar.dma_start(out=bqk_sb[:], in_=bqk)
        nc.scalar.dma_start(out=wk_sb[:], in_=wk.rearrange("(k p) c -> p k c", p=P))
        nc.scalar.dma_start(out=tri_sb[:], in_=tri)
        nc.scalar.dma_start(out=wv_sb[:], in_=wv.rearrange("(k p) c -> p k c", p=P))
        nc.scalar.dma_start(out=w2_sb[:], in_=w2.rearrange("(k p) e -> p k e", p=P))
        nc.sync.dma_start(out=xt_sb[:, :, QCHUNK:S], in_=xTr[:, :, QCHUNK:S].rearrange("k p s -> p k s"))

        psA = ex.enter_context(tc.tile_pool(name="psA", bufs=2, space="PSUM"))
        psS = ex.enter_context(tc.tile_pool(name="psS", bufs=2, space="PSUM"))
        psP = ex.enter_context(tc.tile_pool(name="psP", bufs=1, space="PSUM"))
        psY = ex.enter_context(tc.tile_pool(name="psY", bufs=1, space="PSUM"))
        atp = ex.enter_context(tc.tile_pool(name="atp", bufs=16))
        c2p = ex.enter_context(tc.tile_pool(name="c2p", bufs=10))
        recp = ex.enter_context(tc.tile_pool(name="recp", bufs=4))
        ysbp = ex.enter_context(tc.tile_pool(name="ysbp", bufs=2))

        for c in range(NQC):
            qsl = slice(c * QCHUNK, (c + 1) * QCHUNK)

            # ---- A(c): QKV projection for this query chunk ----
            for p in range(NPAIR):
                for (wsb, dstT, bcol) in ((wq_sb, qT2[p], p),
                                          (wk_sb, kT2[p], NPAIR + p)):
                    ps = psA.tile([P, QCHUNK], FP, tag="psa", name="ps_qk")
                    for k in range(KT):
                        nc.tensor.matmul(
                            ps[:], wsb[:, k, p * P:(p + 1) * P],
                            xt_sb[:, k, qsl],
                            start=(k == 0), stop=(k == KT - 1))
                    nc.vector.tensor_scalar_add(
                        dstT[:, qsl], ps[:], bqk_sb[:, bcol:bcol + 1])
            for t in range(4):
                tt = 4 * c + t
                ps = psA.tile([P, QCHUNK], FP, tag="psa", name="ps_v")
                for k in range(KT):
                    nc.tensor.matmul(
                        ps[:, 0:LC], xt_sb[:, k, tt * P:(tt + 1) * P],
                        wv_sb[:, k, :],
                        start=(k == 0), stop=(k == KT - 1))
                nc.vector.tensor_copy(
                    vall[:, tt, :, 0:D],
                    ps[:, 0:LC].rearrange("p (h d) -> p h d", h=LH))

            # ---- B(c): attention per head ----
            # score tiles are produced in [128, 1024] PSUM pairs (2 key
            # tiles side by side) so one ACT exp covers both.
            c2t = [[None] * 4 for _ in range(NPAIR)]
            for h in range(LH):
                p, half = divmod(h, 2)
                dr = slice(D * half, D * (half + 1))
                at_t = []     # per kt: (tile, col offset)
                at_map = {}   # diagonal kts
                for j in range(2 * c + 2):
                    ps2 = psS.tile([P, 2 * QCHUNK], FP, tag="pss", name="ps_s")
                    at2 = atp.tile([P, 2 * QCHUNK], BF, tag="at", name="at")
                    if j < 2 * c:
                        # both key tiles fully below the diagonal
                        for sub in range(2):
                            kt = 2 * j + sub
                            nc.tensor.matmul(
                                ps2[:, sub * QCHUNK:(sub + 1) * QCHUNK],
                                kT2[p][dr, kt * P:(kt + 1) * P],
                                qT2[p][dr, qsl],
                                start=True, stop=True)
                        nc.scalar.activation(at2[:], ps2[:], EXP, scale=0.125)
                        at_t.append((at2, 0))
                        at_t.append((at2, QCHUNK))
                        continue
                    else:
                        # diagonal "butterfly" pair: key tiles (3,0) resp.
                        # (2,1) of the diagonal block share one [128,1024]
                        # tile so the trimmed regions pack with minimal dead
                        # space.  One exp covers both; the boundary regions
                        # (and the pair-B dead gap) are zeroed below.
                        t = j - 2 * c  # 0: kts (+3, +0);  1: kts (+2, +1)
                        ka, kb = 4 * c + 3 - t, 4 * c + t
                        da, db = (3 - t) * P, t * P
                        nc.tensor.matmul(
                            ps2[:, da:QCHUNK],
                            kT2[p][dr, ka * P:(ka + 1) * P],
                            qT2[p][dr, c * QCHUNK + da:(c + 1) * QCHUNK],
                            start=True, stop=True)
                        nc.tensor.matmul(
                            ps2[:, QCHUNK + db:2 * QCHUNK],
                            kT2[p][dr, kb * P:(kb + 1) * P],
                            qT2[p][dr, c * QCHUNK + db:(c + 1) * QCHUNK],
                            start=True, stop=True)
                        nc.scalar.activation(
                            at2[:, da:2 * QCHUNK], ps2[:, da:2 * QCHUNK],
                            EXP, scale=0.125)
                        nc.vector.copy_predicated(
                            at2[:, da:da + P], tri_sb[:, 0:P],
                            zero_sb[:, 0:P])
                        w = db + P
                        nc.vector.copy_predicated(
                            at2[:, QCHUNK:QCHUNK + w],
                            tri_sb[:, t * P:t * P + w],
                            zero_sb[:, 0:w])
                        at_map[ka] = (at2, 0)
                        at_map[kb] = (at2, QCHUNK)
                po = psP.tile([P, 4, P], FP, tag="po", name="po")
                for qt in range(4):
                    gq = 4 * c + qt
                    for kt in range(gq + 1):
                        att, off = at_t[kt] if kt < 4 * c else at_map[kt]
                        nc.tensor.matmul(
                            po[:, qt, 0:D + 1],
                            att[:, off + qt * P:off + (qt + 1) * P],
                            vall[:, kt, h, :],
                            start=(kt == 0), stop=(kt == gq))
                for qt in range(4):
                    if half == 0:
                        c2t[p][qt] = c2p.tile([P, P], BF, tag="c2", name="c2")
                    rec = recp.tile([P, 1], FP, tag="rec", name="rec")
                    nc.vector.reciprocal(rec[:], po[:, qt, D:D + 1])
                    nc.vector.tensor_scalar_mul(
                        c2t[p][qt][:, half * D:(half + 1) * D],
                        po[:, qt, 0:D], rec[:])
                    if half == 1:
                        # both heads of the pair done: transpose to cT
                        tt = 4 * c + qt
                        nc.sync.dma_start_transpose(
                            cT[p][:, tt * P:(tt + 1) * P], c2t[p][qt][:])

            # ---- C(c): partial c_proj + output DMA ----
            # (for the last chunk, B is winding down: use the wide psS slots
            # so both E-halves pipeline and evict in one DVE op)
            for t in range(4):
                tt = 4 * c + t
                ysb = ysbp.tile([P, E], FP, tag="ysb", name="ysb")
                if c == NQC - 1:
                    ps2y = psS.tile([P, 2 * QCHUNK], FP, tag="pss", name="ps_y2")
                    for e in range(2):
                        for ct in range(NPAIR):
                            nc.tensor.matmul(
                                ps2y[:, e * QCHUNK:(e + 1) * QCHUNK],
                                cT[ct][:, tt * P:(tt + 1) * P],
                                w2_sb[:, ct, e * QCHUNK:(e + 1) * QCHUNK],
                                start=(ct == 0), stop=(ct == NPAIR - 1))
                    nc.vector.tensor_copy(ysb[:], ps2y[:])
                else:
                    for e in range(2):
                        ps = psY.tile([P, QCHUNK], FP, tag="psy", name="ps_y")
                        for ct in range(NPAIR):
                            nc.tensor.matmul(
                                ps[:], cT[ct][:, tt * P:(tt + 1) * P],
                                w2_sb[:, ct, e * QCHUNK:(e + 1) * QCHUNK],
                                start=(ct == 0), stop=(ct == NPAIR - 1))
                        nc.vector.tensor_copy(
                            ysb[:, e * QCHUNK:(e + 1) * QCHUNK], ps[:])
                nc.sync.dma_start(out=y[tt * P:(tt + 1) * P, :], in_=ysb[:])


_module = None


def _get_module():
    global _module
    if _module is None:
        _module = _build_module()
    return _module


def _make_tri():
    # boundary-zeroing predicates (1 = masked-out): cols 0:128 = (j < k),
    # cols 128:384 = (j < 128+k) (dead gap + boundary of a second-half tile
    # whose causal delta is one key-tile above the half boundary).
    i = np.arange(P)[:, None]
    m1 = (np.arange(P)[None, :] < i)
    m2 = (np.arange(2 * P)[None, :] < P + i)
    return np.concatenate([m1, m2], axis=1).astype(np.uint8)


def kernel(hidden_states, c_attn_w, c_attn_b, c_proj_w, c_proj_b):
    hidden_states = np.asarray(hidden_states, np.float32)
    c_attn_w = np.asarray(c_attn_w, np.float32)
    c_attn_b = np.asarray(c_attn_b, np.float32)
    c_proj_w = np.asarray(c_proj_w, np.float32)
    c_proj_b = np.asarray(c_proj_b, np.float32)

    nc = _get_module()
    tri = _make_tri()
    in_maps = []
    for core in range(NCORES):
        b, g = divmod(core, HG)
        cols = slice(g * LC, (g + 1) * LC)
        # bias columns: [q pair0, q pair1, k pair0, k pair1]
        bias_cols = np.stack(
            [c_attn_b[0 * E + g * LC + p * P: 0 * E + g * LC + (p + 1) * P]
             for p in range(NPAIR)] +
            [c_attn_b[1 * E + g * LC + p * P: 1 * E + g * LC + (p + 1) * P]
             for p in range(NPAIR)], axis=1)
        in_maps.append({
            "xT": np.ascontiguousarray(hidden_states[b].T).astype(ml_dtypes.bfloat16),
            "wq": np.ascontiguousarray(c_attn_w[:, 0 * E:1 * E][:, cols]).astype(ml_dtypes.bfloat16),
            "wk": np.ascontiguousarray(c_attn_w[:, 1 * E:2 * E][:, cols]).astype(ml_dtypes.bfloat16),
            "wv": np.ascontiguousarray(c_attn_w[:, 2 * E:3 * E][:, cols]).astype(ml_dtypes.bfloat16),
            "w2": np.ascontiguousarray(c_proj_w[cols, :]).astype(ml_dtypes.bfloat16),
            "tri": tri,
            "bqk": np.ascontiguousarray(bias_cols),
        })

    global _last_in_maps
    _last_in_maps = in_maps
    res = bass_utils.run_bass_kernel_spmd(
        nc, in_maps, core_ids=list(range(NCORES)))

    # v-bias folds through attention (rows sum to 1): + bv @ Wproj + bproj
    bias_out = c_attn_b[2 * E:3 * E] @ c_proj_w + c_proj_b
    out = np.empty((B, S, E), np.float32)
    for b in range(B):
        acc = res.results[b * HG + 0]["y"].astype(np.float32).copy()
        for g in range(1, HG):
            acc += res.results[b * HG + g]["y"]
        out[b] = acc + bias_out
    return out


# revision 11
# speedup vs baseline: 1.0982x; 1.0982x over previous
"""GPT2Attention Trainium2 Bass kernel.

Problem: B=2, S=2048, E=1024, H=16 heads, D=64.
  qkv = x @ c_attn_w + c_attn_b; causal softmax attention; out @ c_proj_w + c_proj_b.

Sharding: 8 cores = 2 (batch) x 4 (head-groups of 4 heads).  Each core computes
its batch's attention for its 4 heads plus the partial c_proj contribution
(rows of c_proj_w belonging to its heads).  Host sums the 4 partials per batch
and adds the bias terms (v-bias folds through attention: attn rows sum to 1).

Device pipeline (chunked over 512-query blocks so the tile scheduler can
overlap the PE-heavy projections with the ACT-heavy softmax):
  A(c): QKV projection for query chunk c.  qT/kT per head-pair [128, S]
        (partitions = 2 heads x 64 dims); v natural [tokens, 4 heads, 65]
        with a ones column for the softmax denominator.
  B(c): per head: scoresT tiles [128 keys, 512 q] on PE (trimmed to the
        causal boundary), exp on ACT, diagonal boundary zeroed via
        copy_predicated on DVE.  attn@v runs transposed: po[128 q, 65]
        accumulates over key tiles (cost = 65 rows/matmul instead of 512).
        Normalize by the ones-column denominator -> c2 [128 tok, 128 dims].
  T(c): DMA-transpose c2 -> cT [dims, tokens] (idle DMA hardware).
  C(c): partial c_proj from cT; evict PSUM->SBUF split ACT/DVE; DMA y out.
"""

from contextlib import ExitStack

import numpy as np
import ml_dtypes

import bass_rust
import concourse.bass as bass
import concourse.tile as tile
from concourse import mybir
from concourse import bass_utils


def _patched_drain_and_barrier(self, tick_clock, wait_clock):
    # The stock walrus in this container rejects instructions carrying more
    # than one sync wait ("Too many sync wait commands" on the kernel-tail
    # Drain).  Spread the final waits across single-wait NOPs instead.
    nc = self.nc
    probe = nc.sync.nop()
    wait_clock.add_sem_waits(
        probe.ins, bass_rust.ScopedClock({None: tick_clock.global_clock}))
    si = probe.ins.sync_info
    waits = list(si.on_wait) if si is not None else []
    if len(waits) > 1:
        probe.ins.sync_info = mybir.SyncInfo(
            on_wait=waits[:1], on_update=list(si.on_update))
        for w in waits[1:]:
            n = nc.sync.nop()
            n.ins.sync_info = mybir.SyncInfo(on_wait=[w], on_update=[])
    nc.sync.drain()
    nc.all_engine_barrier()
    assert self.sems is not None
    popped = nc._tile_sem_poison_stack.pop()
    assert popped is self._sem_poison
    nc.clear_and_free_semaphores(list(self.sems.allocated().values()))
    nc.all_engine_barrier()


tile.TileContext._drain_and_barrier = _patched_drain_and_barrier

_split_ctr = [0]


def _split_sync_waits(nc):
    """Stock walrus allows one sync wait per instruction; hoist extras onto
    single-wait NOPs inserted just before, on the same (in-order) engine."""
    for fn in nc.m.functions:
        for bb in fn.blocks:
            insts = bb.instructions
            out = []
            changed = False
            for inst in insts:
                si = getattr(inst, "sync_info", None)
                waits = list(si.on_wait) if si is not None else []
                if len(waits) > 1:
                    changed = True
                    for w in waits[:-1]:
                        _split_ctr[0] += 1
                        nop = bass_rust.InstNoOp(
                            name=f"I-syncsplit-{_split_ctr[0]}",
                            engine=inst.engine)
                        nop.sync_info = mybir.SyncInfo(on_wait=[w], on_update=[])
                        out.append(nop)
                    inst.sync_info = mybir.SyncInfo(
                        on_wait=[waits[-1]], on_update=list(si.on_update))
                out.append(inst)
            if changed:
                bb.instructions = out

B, S, E, H, D = 2, 2048, 1024, 16, 64
NCORES = 8
HG = 4                # head-group cores per batch
LH = H // HG          # 4 local heads per core
LC = LH * D           # 256 local c_proj rows
NPAIR = LH // 2       # 2 head-pairs per core
P = 128
KT = E // P           # 8 contraction tiles for the projections
QCHUNK = 512
NQC = S // QCHUNK     # 4 query chunks
NKT = S // P          # 16 key tiles
NTT = S // P          # 16 token tiles

FP = mybir.dt.float32
BF = mybir.dt.bfloat16
EXP = mybir.ActivationFunctionType.Exp
COPY = mybir.ActivationFunctionType.Copy


def _build_module():
    nc = bass.Bass("TRN2", target_bir_lowering=False, debug=False,
                   num_devices=NCORES)
    xT = nc.dram_tensor("xT", [E, S], BF, kind="ExternalInput").ap()
    wq = nc.dram_tensor("wq", [E, LC], BF, kind="ExternalInput").ap()
    wk = nc.dram_tensor("wk", [E, LC], BF, kind="ExternalInput").ap()
    wv = nc.dram_tensor("wv", [E, LC], BF, kind="ExternalInput").ap()
    w2 = nc.dram_tensor("w2", [LC, E], BF, kind="ExternalInput").ap()
    tri = nc.dram_tensor("tri", [P, 3 * P], mybir.dt.uint8,
                         kind="ExternalInput").ap()
    bqk = nc.dram_tensor("bqk", [P, 2 * NPAIR], FP, kind="ExternalInput").ap()
    y = nc.dram_tensor("y", [S, E], FP, kind="ExternalOutput").ap()

    with tile.TileContext(nc) as tc:
        _body(tc, xT, wq, wk, wv, w2, tri, bqk, y)
    _split_sync_waits(nc)
    return nc


def _body(tc, xT, wq, wk, wv, w2, tri, bqk, y):
    nc = tc.nc
    ex = ExitStack()
    with ex:
        persist = ex.enter_context(tc.tile_pool(name="persist", bufs=1))

        # ---- persistent tiles ----
        qT2 = [persist.tile([P, S], BF, name=f"qT2_{p}") for p in range(NPAIR)]
        kT2 = [persist.tile([P, S], BF, name=f"kT2_{p}") for p in range(NPAIR)]
        # v natural layout: [token-partitions, ttile, head, dim(+denom col)]
        vall = persist.tile([P, NTT, LH, D + 1], BF, name="vall")
        cT = [persist.tile([P, S], BF, name=f"cT_{p}") for p in range(NPAIR)]
        xt_sb = persist.tile([P, KT, S], BF, name="xt_sb")
        wq_sb = persist.tile([P, KT, LC], BF, name="wq_sb")
        wk_sb = persist.tile([P, KT, LC], BF, name="wk_sb")
        wv_sb = persist.tile([P, KT, LC], BF, name="wv_sb")
        w2_sb = persist.tile([P, 2, E], BF, name="w2_sb")
        # masking predicates for the causal boundary: cols 0:128 = (j < k),
        # 128:384 = (j < 128+k)
        tri_sb = persist.tile([P, 3 * P], mybir.dt.uint8, name="tri_sb")
        zero_sb = persist.tile([P, 2 * P], BF, name="zero_sb")
        bqk_sb = persist.tile([P, 2 * NPAIR], FP, name="bqk_sb")

        nc.vector.memset(zero_sb[:], 0.0)
        nc.vector.memset(vall[:, :, :, D:D + 1], 1.0)

        # ---- input DMAs (ordered so chunk-0 work can start early) ----
        xTr = xT.rearrange("(k p) s -> k p s", p=P)
        nc.scalar.dma_start(out=wq_sb[:], in_=wq.rearrange("(k p) c -> p k c", p=P))
        nc.sync.dma_start(out=xt_sb[:, :, 0:QCHUNK],
                          in_=xTr[:, :, 0:QCHUNK].rearrange("k p s -> p k s"))
        nc.scalar.dma_start(out=wk_sb[:], in_=wk.rearrange("(k p) c -> p k c", p=P))
        nc.scalar.dma_start(out=bqk_sb[:], in_=bqk)
        nc.scalar.dma_start(out=tri_sb[:], in_=tri)
        nc.scalar.dma_start(out=wv_sb[:], in_=wv.rearrange("(k p) c -> p k c", p=P))
        for cc in range(1, NQC):
            csl = slice(cc * QCHUNK, (cc + 1) * QCHUNK)
            nc.sync.dma_start(out=xt_sb[:, :, csl],
                              in_=xTr[:, :, csl].rearrange("k p s -> p k s"))
            if cc == 1:
                nc.scalar.dma_start(
                    out=w2_sb[:], in_=w2.rearrange("(k p) e -> p k e", p=P))

        psA = ex.enter_context(tc.tile_pool(name="psA", bufs=2, space="PSUM"))
        psS = ex.enter_context(tc.tile_pool(name="psS", bufs=2, space="PSUM"))
        psP = ex.enter_context(tc.tile_pool(name="psP", bufs=1, space="PSUM"))
        psY = ex.enter_context(tc.tile_pool(name="psY", bufs=1, space="PSUM"))
        atp = ex.enter_context(tc.tile_pool(name="atp", bufs=16))
        c2p = ex.enter_context(tc.tile_pool(name="c2p", bufs=10))
        recp = ex.enter_context(tc.tile_pool(name="recp", bufs=4))
        ysbp = ex.enter_context(tc.tile_pool(name="ysbp", bufs=2))

        for c in range(NQC):
            qsl = slice(c * QCHUNK, (c + 1) * QCHUNK)

            # ---- A(c): QKV projection for this query chunk ----
            for p in range(NPAIR):
                for (wsb, dstT, bcol) in ((wq_sb, qT2[p], p),
                                          (wk_sb, kT2[p], NPAIR + p)):
                    ps = psA.tile([P, QCHUNK], FP, tag="psa", name="ps_qk")
                    for k in range(KT):
                        nc.tensor.matmul(
                            ps[:], wsb[:, k, p * P:(p + 1) * P],
                            xt_sb[:, k, qsl],
                            start=(k == 0), stop=(k == KT - 1))
                    nc.vector.tensor_scalar_add(
                        dstT[:, qsl], ps[:], bqk_sb[:, bcol:bcol + 1])
            for t in range(4):
                tt = 4 * c + t
                ps = psA.tile([P, QCHUNK], FP, tag="psa", name="ps_v")
                for k in range(KT):
                    nc.tensor.matmul(
                        ps[:, 0:LC], xt_sb[:, k, tt * P:(tt + 1) * P],
                        wv_sb[:, k, :],
                        start=(k == 0), stop=(k == KT - 1))
                nc.vector.tensor_copy(
                    vall[:, tt, :, 0:D],
                    ps[:, 0:LC].rearrange("p (h d) -> p h d", h=LH))

            # ---- B(c): attention per head ----
            # score tiles are produced in [128, 1024] PSUM pairs (2 key
            # tiles side by side) so one ACT exp covers both.
            c2t = [[None] * 4 for _ in range(NPAIR)]
            for h in range(LH):
                p, half = divmod(h, 2)
                dr = slice(D * half, D * (half + 1))
                at_t = []     # per kt: (tile, col offset)
                at_map = {}   # diagonal kts
                for j in range(2 * c + 2):
                    ps2 = psS.tile([P, 2 * QCHUNK], FP, tag="pss", name="ps_s")
                    at2 = atp.tile([P, 2 * QCHUNK], BF, tag="at", name="at")
                    if j < 2 * c:
                        # both key tiles fully below the diagonal
                        for sub in range(2):
                            kt = 2 * j + sub
                            nc.tensor.matmul(
                                ps2[:, sub * QCHUNK:(sub + 1) * QCHUNK],
                                kT2[p][dr, kt * P:(kt + 1) * P],
                                qT2[p][dr, qsl],
                                start=True, stop=True)
                        nc.scalar.activation(at2[:], ps2[:], EXP, scale=0.125)
                        at_t.append((at2, 0))
                        at_t.append((at2, QCHUNK))
                        continue
                    else:
                        # diagonal "butterfly" pair: key tiles (3,0) resp.
                        # (2,1) of the diagonal block share one [128,1024]
                        # tile so the trimmed regions pack with minimal dead
                        # space.  One exp covers both; the boundary regions
                        # (and the pair-B dead gap) are zeroed below.
                        t = j - 2 * c  # 0: kts (+3, +0);  1: kts (+2, +1)
                        ka, kb = 4 * c + 3 - t, 4 * c + t
                        da, db = (3 - t) * P, t * P
                        nc.tensor.matmul(
                            ps2[:, da:QCHUNK],
                            kT2[p][dr, ka * P:(ka + 1) * P],
                            qT2[p][dr, c * QCHUNK + da:(c + 1) * QCHUNK],
                            start=True, stop=True)
                        nc.tensor.matmul(
                            ps2[:, QCHUNK + db:2 * QCHUNK],
                            kT2[p][dr, kb * P:(kb + 1) * P],
                            qT2[p][dr, c * QCHUNK + db:(c + 1) * QCHUNK],
                            start=True, stop=True)
                        nc.scalar.activation(
                            at2[:, da:2 * QCHUNK], ps2[:, da:2 * QCHUNK],
                            EXP, scale=0.125)
                        nc.vector.copy_predicated(
                            at2[:, da:da + P], tri_sb[:, 0:P],
                            zero_sb[:, 0:P])
                        w = db + P
                        nc.vector.copy_predicated(
                            at2[:, QCHUNK:QCHUNK + w],
                            tri_sb[:, t * P:t * P + w],
                            zero_sb[:, 0:w])
                        at_map[ka] = (at2, 0)
                        at_map[kb] = (at2, QCHUNK)
                po = psP.tile([P, 4, P], FP, tag="po", name="po")
                for qt in range(4):
                    gq = 4 * c + qt
                    for kt in range(gq + 1):
                        att, off = at_t[kt] if kt < 4 * c else at_map[kt]
                        nc.tensor.matmul(
                            po[:, qt, 0:D + 1],
                            att[:, off + qt * P:off + (qt + 1) * P],
                            vall[:, kt, h, :],
                            start=(kt == 0), stop=(kt == gq))
                for qt in range(4):
                    if half == 0:
                        c2t[p][qt] = c2p.tile([P, P], BF, tag="c2", name="c2")
                    rec = recp.tile([P, 1], FP, tag="rec", name="rec")
                    nc.vector.reciprocal(rec[:], po[:, qt, D:D + 1])
                    nc.vector.tensor_scalar_mul(
                        c2t[p][qt][:, half * D:(half + 1) * D],
                        po[:, qt, 0:D], rec[:])
                    if half == 1:
                        # both heads of the pair done: transpose to cT
                        tt = 4 * c + qt
                        nc.sync.dma_start_transpose(
                            cT[p][:, tt * P:(tt + 1) * P], c2t[p][qt][:])

            # ---- C(c): partial c_proj + output DMA ----
            # (for the last chunk, B is winding down: use the wide psS slots
            # so both E-halves pipeline and evict in one DVE op)
            for t in range(4):
                tt = 4 * c + t
                ysb = ysbp.tile([P, E], FP, tag="ysb", name="ysb")
                if c == NQC - 1:
                    ps2y = psS.tile([P, 2 * QCHUNK], FP, tag="pss", name="ps_y2")
                    for e in range(2):
                        for ct in range(NPAIR):
                            nc.tensor.matmul(
                                ps2y[:, e * QCHUNK:(e + 1) * QCHUNK],
                                cT[ct][:, tt * P:(tt + 1) * P],
                                w2_sb[:, ct, e * QCHUNK:(e + 1) * QCHUNK],
                                start=(ct == 0), stop=(ct == NPAIR - 1))
                    nc.vector.tensor_copy(ysb[:], ps2y[:])
                else:
                    for e in range(2):
                        ps = psY.tile([P, QCHUNK], FP, tag="psy", name="ps_y")
                        for ct in range(NPAIR):
                            nc.tensor.matmul(
                                ps[:], cT[ct][:, tt * P:(tt + 1) * P],
                                w2_sb[:, ct, e * QCHUNK:(e + 1) * QCHUNK],
                                start=(ct == 0), stop=(ct == NPAIR - 1))
                        nc.vector.tensor_copy(
                            ysb[:, e * QCHUNK:(e + 1) * QCHUNK], ps[:])
                nc.sync.dma_start(out=y[tt * P:(tt + 1) * P, :], in_=ysb[:])


_module = None


def _get_module():
    global _module
    if _module is None:
        _module = _build_module()
    return _module


def _make_tri():
    # boundary-zeroing predicates (1 = masked-out): cols 0:128 = (j < k),
    # cols 128:384 = (j < 128+k) (dead gap + boundary of a second-half tile
    # whose causal delta is one key-tile above the half boundary).
    i = np.arange(P)[:, None]
    m1 = (np.arange(P)[None, :] < i)
    m2 = (np.arange(2 * P)[None, :] < P + i)
    return np.concatenate([m1, m2], axis=1).astype(np.uint8)


def kernel(hidden_states, c_attn_w, c_attn_b, c_proj_w, c_proj_b):
    hidden_states = np.asarray(hidden_states, np.float32)
    c_attn_w = np.asarray(c_attn_w, np.float32)
    c_attn_b = np.asarray(c_attn_b, np.float32)
    c_proj_w = np.asarray(c_proj_w, np.float32)
    c_proj_b = np.asarray(c_proj_b, np.float32)

    nc = _get_module()
    tri = _make_tri()
    in_maps = []
    for core in range(NCORES):
        b, g = divmod(core, HG)
        cols = slice(g * LC, (g + 1) * LC)
        # bias columns: [q pair0, q pair1, k pair0, k pair1]
        bias_cols = np.stack(
            [c_attn_b[0 * E + g * LC + p * P: 0 * E + g * LC + (p + 1) * P]
             for p in range(NPAIR)] +
            [c_attn_b[1 * E + g * LC + p * P: 1 * E + g * LC + (p + 1) * P]
             for p in range(NPAIR)], axis=1)
        in_maps.append({
            "xT": np.ascontiguousarray(hidden_states[b].T).astype(ml_dtypes.bfloat16),
            "wq": np.ascontiguousarray(c_attn_w[:, 0 * E:1 * E][:, cols]).astype(ml_dtypes.bfloat16),
            "wk": np.ascontiguousarray(c_attn_w[:, 1 * E:2 * E][:, cols]).astype(ml_dtypes.bfloat16),
            "wv": np.ascontiguousarray(c_attn_w[:, 2 * E:3 * E][:, cols]).astype(ml_dtypes.bfloat16),
            "w2": np.ascontiguousarray(c_proj_w[cols, :]).astype(ml_dtypes.bfloat16),
            "tri": tri,
            "bqk": np.ascontiguousarray(bias_cols),
        })

    global _last_in_maps
    _last_in_maps = in_maps
    res = bass_utils.run_bass_kernel_spmd(
        nc, in_maps, core_ids=list(range(NCORES)))

    # v-bias folds through attention (rows sum to 1): + bv @ Wproj + bproj
    bias_out = c_attn_b[2 * E:3 * E] @ c_proj_w + c_proj_b
    out = np.empty((B, S, E), np.float32)
    for b in range(B):
        acc = res.results[b * HG + 0]["y"].astype(np.float32).copy()
        for g in range(1, HG):
            acc += res.results[b * HG + g]["y"]
        out[b] = acc + bias_out
    return out
